# revision 7
# baseline (speedup 1.0000x reference)
"""Distributed Bass kernel for a dense-transformer attention layer on 8 TRN2 cores.

Strategy (tensor-parallel over heads, Megatron-style, zero on-device transposes):
  - Host: transpose hidden -> xT [H, T]; per-core column-shard of Wq/Wk (rows
    permuted per head so rotary pairs land in partition halves), Wv; full Wd.T.
  - Device, per core (SPMD, 2 heads each):
      A) QKV projections:  Q_T/K_T produced directly in [head_dim, tokens]
         layout (weights stationary, xT streamed);  V in [tokens, head_dim]
         layout (xT stationary).  Results spilled to DRAM scratch.
      B) Attention per (batch, head): load Q_T/K_T head tile plus a partition-
         half-swapped copy (DMA), apply RoPE with three partition-aligned DVE
         ops using host cos/sin tables.  Scores computed TRANSPOSED
         S_T[kpos, q] = K_T_tile(lhsT) @ Q_T block, causal block-skipping,
         additive mask bias for diagonal blocks, exp without max-subtraction
         (score magnitudes are tiny), softmax denominator via ones-matmul
         (which also yields the partition-broadcast for free), context
         accumulated as ctx_T[d, q] += V_tile(lhsT) @ P_T in PSUM.
      C) AllToAll of ctx features -> each core holds the full 2048 ctx features
         for its 512-token block; output projection out = ctx_T.T @ Wd.T.
  - Host: concat the 8 token blocks -> [B, S, H].
"""

import os
import sys
import math
from dataclasses import dataclass, field

import numpy as np

sys.path.insert(0, "/opt/trn_rl_repo")

# ---------------------------------------------------------------- problem dims
B, S, H, NH = 2, 2048, 2048, 16
HD = H // NH  # 128
NCORES = 8
ROPE_BASE = 10000.0
SCALE = 1.0 / math.sqrt(HD)
NEG_BIAS = -1.0e6  # additive mask bias; exp(SCALE * (x + NEG_BIAS)) == 0.0f

KB = 128  # key-block (kpos per score tile)
QB = 512  # query-block (free dim of score tiles) == tokens per core block
OB = 512  # output-projection n-chunk

LAST_EXEC_NS = None  # set when BASS_KERNEL_TRACE=1


@dataclass
class Config:
    b: int = B
    s: int = S
    h: int = H
    nh: int = NH
    ncores: int = NCORES
    qb: int = QB
    ob: int = OB
    qbp: int = 512  # token-chunk width for the projection stage
    use_fp32r: bool = True
    # block_map[jq][kb] = "skip" | "free" | bias-tile index (int)
    block_map: list = field(default_factory=list)
    n_bias: int = 0

    @property
    def hd(self):
        return self.h // self.nh

    @property
    def t(self):
        return self.b * self.s

    @property
    def hpc(self):  # heads per core
        return self.nh // self.ncores

    @property
    def f(self):  # features per core
        return self.hpc * self.hd

    @property
    def nb(self):  # tokens per core output block
        return self.t // self.ncores


def classify_blocks(mask2d: np.ndarray, qb: int, kb: int):
    """mask2d: [S, S] bool, True = masked out.  Returns (block_map, bias_tiles).

    block_map[jq][k] in {"skip", "free", int}; bias_tiles[i] is [KB, QB] f32
    (transposed: [kpos, q]) with NEG_BIAS where masked.
    """
    s = mask2d.shape[0]
    bias_tiles = []
    bias_keys = {}
    block_map = []
    for jq in range(s // qb):
        row = []
        for k in range(s // kb):
            sub = mask2d[jq * qb:(jq + 1) * qb, k * kb:(k + 1) * kb]
            if sub.all():
                row.append("skip")
            elif not sub.any():
                row.append("free")
            else:
                tile = np.where(sub.T, np.float32(NEG_BIAS), np.float32(0.0))
                key = tile.tobytes()
                if key not in bias_keys:
                    bias_keys[key] = len(bias_tiles)
                    bias_tiles.append(np.ascontiguousarray(tile, np.float32))
                row.append(bias_keys[key])
        assert any(c != "skip" for c in row), "fully-masked query block"
        block_map.append(row)
    return block_map, bias_tiles


# ------------------------------------------------------------------ host sharding
def prepare(hidden_states, Wq, Wk, Wv, Wd, attention_mask, position_ids, cfg):
    s, h, nh, hd = cfg.s, cfg.h, cfg.nh, cfg.hd
    t = cfg.t

    x = np.ascontiguousarray(np.asarray(hidden_states, np.float32).reshape(t, h))
    xT = np.ascontiguousarray(x.T)  # [H, T]

    # per-head pair permutation: [0,2,...,hd-2, 1,3,...,hd-1]
    pp = np.concatenate([np.arange(0, hd, 2), np.arange(1, hd, 2)])
    perm = np.concatenate([hh * hd + pp for hh in range(nh)])

    WqP = np.asarray(Wq, np.float32)[perm]
    WkP = np.asarray(Wk, np.float32)[perm]
    Wv_ = np.asarray(Wv, np.float32)
    WdT = np.ascontiguousarray(np.asarray(Wd, np.float32).T)  # [H(in f), H(out)]

    # rotary tables in permuted layout: rows [0:hd/2] pair-first, [hd/2:hd] pair-second
    inv_freq = (1.0 / (ROPE_BASE ** (np.arange(0, hd, 2, dtype=np.float32) / np.float32(hd)))).astype(np.float32)
    pos = np.asarray(position_ids).astype(np.float32).reshape(t)  # [T]
    ang = pos[None, :] * inv_freq[:, None]  # [hd/2, T]
    cos = np.cos(ang).astype(np.float32)
    sin = np.sin(ang).astype(np.float32)
    cosT = np.ascontiguousarray(np.concatenate([cos, cos], axis=0))  # [hd, T]
    sinT = np.ascontiguousarray(np.concatenate([-sin, sin], axis=0))  # [hd, T]

    mask2d = np.asarray(attention_mask).reshape(s, s).astype(bool)
    cfg.block_map, bias_tiles = classify_blocks(mask2d, cfg.qb, KB)
    cfg.n_bias = len(bias_tiles)
    maskb = None
    if cfg.n_bias:
        maskb = np.ascontiguousarray(np.stack(bias_tiles, axis=0))

    in_maps = []
    f = cfg.f
    for c in range(cfg.ncores):
        m = {
            "xT": xT,
            "wqT": np.ascontiguousarray(WqP[c * f:(c + 1) * f].T),  # [H, F]
            "wkT": np.ascontiguousarray(WkP[c * f:(c + 1) * f].T),
            "wvT": np.ascontiguousarray(Wv_[c * f:(c + 1) * f].T),
            "wdT": WdT,
            "cosT": cosT,
            "sinT": sinT,
            "ones": np.ones((128, 128), np.float32),
        }
        if cfg.n_bias:
            m["maskb"] = maskb
        in_maps.append(m)
    return in_maps


# ------------------------------------------------------------------ graph builder
def build_graph(cfg):
    import concourse.tile as tile
    from concourse import bacc, mybir

    dt = mybir.dt
    mmdt = dt.float32r if cfg.use_fp32r else dt.float32

    b, s, h = cfg.b, cfg.s, cfg.h
    t, f, hd, hpc = cfg.t, cfg.f, cfg.hd, cfg.hpc
    nb, qb, ob, qbp = cfg.nb, cfg.qb, cfg.ob, cfg.qbp
    nkt = h // 128          # k-tiles over hidden dim
    npc = t // qbp          # token chunks in projection stage
    hh = hd // 2
    assert qb == nb, (qb, nb)
    assert hd == 128

    nc = bacc.Bacc(None, target_bir_lowering=False)

    xT = nc.declare_dram_parameter("xT", [h, t], dt.float32, isOutput=False)
    wqT = nc.declare_dram_parameter("wqT", [h, f], dt.float32, isOutput=False)
    wkT = nc.declare_dram_parameter("wkT", [h, f], dt.float32, isOutput=False)
    wvT = nc.declare_dram_parameter("wvT", [h, f], dt.float32, isOutput=False)
    wdT = nc.declare_dram_parameter("wdT", [h, h], dt.float32, isOutput=False)
    cosT = nc.declare_dram_parameter("cosT", [hd, t], dt.float32, isOutput=False)
    sinT = nc.declare_dram_parameter("sinT", [hd, t], dt.float32, isOutput=False)
    ones_d = nc.declare_dram_parameter("ones", [128, 128], dt.float32, isOutput=False)
    if cfg.n_bias:
        maskb = nc.declare_dram_parameter("maskb", [cfg.n_bias, KB, qb], dt.float32, isOutput=False)
    out = nc.declare_dram_parameter("out", [nb, h], dt.float32, isOutput=True)

    def mm(out_ap, lhsT, rhs, start, stop):
        nc.tensor.matmul(out_ap, lhsT, rhs, start=start, stop=stop)

    def bc(ap):  # view a float32 DRAM AP as the matmul dtype
        return ap.bitcast(mmdt)

    with tile.TileContext(nc) as tc:
        with (
            tc.tile_pool(name="persist", bufs=1) as persist,
            tc.tile_pool(name="dram", bufs=1, space="DRAM") as dram,
        ):
            ones_sb = persist.tile([128, 128], mmdt, name="ones_sb")
            nc.sync.dma_start(out=ones_sb[:], in_=bc(ones_d[:, :]))
            mask_sb = []
            for i in range(cfg.n_bias):
                mt = persist.tile([KB, qb], dt.float32, name=f"mask_sb{i}")
                nc.sync.dma_start(out=mt[:], in_=maskb[i, :, :])
                mask_sb.append(mt)

            qt_dram = dram.tile([f, t], dt.float32, name="qt_dram")
            kt_dram = dram.tile([f, t], dt.float32, name="kt_dram")
            v_dram = dram.tile([t, f], dt.float32, name="v_dram")
            a2a_in = dram.tile([cfg.ncores, f, nb], dt.float32, name="a2a_in")
            a2a_out = dram.tile([cfg.ncores, f, nb], dt.float32, name="a2a_out")

            # ---------------- stage A: QKV projections ----------------
            with (
                tc.tile_pool(name="a_w", bufs=1) as a_w,
                tc.tile_pool(name="a_x", bufs=nkt + 8) as a_x,
                tc.tile_pool(name="a_ep", bufs=4) as a_ep,
                tc.tile_pool(name="a_ps", bufs=6, space="PSUM") as a_ps,
            ):
                wq_sb, wk_sb, wv_sb = [], [], []
                for k in range(nkt):
                    for lst, src, nm in (
                        (wq_sb, wqT, "wq"),
                        (wk_sb, wkT, "wk"),
                        (wv_sb, wvT, "wv"),
                    ):
                        tl = a_w.tile([128, f], mmdt, name=f"{nm}_sb{k}")
                        nc.sync.dma_start(out=tl[:], in_=bc(src[k * 128:(k + 1) * 128, :]))
                        lst.append(tl)

                for c in range(npc):
                    c0 = c * qbp
                    xs = []
                    for k in range(nkt):
                        xt = a_x.tile([128, qbp], mmdt, name="xslab", tag="xslab")
                        nc.sync.dma_start(
                            out=xt[:], in_=bc(xT[k * 128:(k + 1) * 128, c0:c0 + qbp])
                        )
                        xs.append(xt)

                    # Q_T and K_T: [f, tokens], weights stationary
                    for w_sb, dst in ((wq_sb, qt_dram), (wk_sb, kt_dram)):
                        for ft in range(f // 128):
                            ps = a_ps.tile([128, qbp], dt.float32, name="proj_ps", tag="proj_ps")
                            for k in range(nkt):
                                mm(ps[:], w_sb[k][:, ft * 128:(ft + 1) * 128], xs[k][:],
                                   start=(k == 0), stop=(k == nkt - 1))
                            qk = a_ep.tile([128, qbp], dt.float32, name="proj_o", tag="proj_o")
                            nc.vector.tensor_copy(qk[:], ps[:])
                            nc.sync.dma_start(
                                out=dst[ft * 128:(ft + 1) * 128, c0:c0 + qbp], in_=qk[:]
                            )

                    # V: [tokens, f], activations stationary
                    for ts in range(qbp // 128):
                        ps = a_ps.tile([128, f], dt.float32, name="v_ps", tag="proj_ps")
                        for k in range(nkt):
                            mm(ps[:], xs[k][:, ts * 128:(ts + 1) * 128], wv_sb[k][:],
                               start=(k == 0), stop=(k == nkt - 1))
                        vsb = a_ep.tile([128, f], dt.float32, name="v_sb", tag="proj_o")
                        nc.vector.tensor_copy(vsb[:], ps[:])
                        nc.sync.dma_start(
                            out=v_dram[c0 + ts * 128:c0 + (ts + 1) * 128, :], in_=vsb[:]
                        )

            # ---------------- stage B: RoPE + attention ----------------
            with (
                tc.tile_pool(name="b_cs", bufs=1) as b_cs,
                tc.tile_pool(name="b_qk", bufs=3) as b_qk,
                tc.tile_pool(name="b_tmp", bufs=2) as b_tmp,
                tc.tile_pool(name="b_v", bufs=24) as b_v,
                tc.tile_pool(name="b_p", bufs=4) as b_p,
                tc.tile_pool(name="b_acc", bufs=3) as b_acc,
                tc.tile_pool(name="b_sps", bufs=3, space="PSUM") as b_sps,
                tc.tile_pool(name="b_cps", bufs=2, space="PSUM") as b_cps,
                tc.tile_pool(name="b_dps", bufs=2, space="PSUM") as b_dps,
            ):
                cos_sb = b_cs.tile([hd, t], mmdt, name="cos_sb")
                sin_sb = b_cs.tile([hd, t], mmdt, name="sin_sb")
                nc.sync.dma_start(out=cos_sb[:], in_=bc(cosT[:, :]))
                nc.sync.dma_start(out=sin_sb[:], in_=bc(sinT[:, :]))

                for bb in range(b):
                    bcol = bb * s
                    for hi in range(hpc):
                        qrow = hi * hd
                        # load head tiles + partition-half-swapped copies, RoPE
                        rot = {}
                        for src, nm in ((qt_dram, "q"), (kt_dram, "k")):
                            tl = b_qk.tile([hd, s], mmdt, name=f"{nm}t_sb", tag=f"{nm}t_sb")
                            sw = b_qk.tile([hd, s], mmdt, name=f"{nm}t_sw", tag=f"{nm}t_sw")
                            nc.sync.dma_start(out=tl[:], in_=bc(src[qrow:qrow + hd, bcol:bcol + s]))
                            nc.sync.dma_start(out=sw[:hh, :], in_=bc(src[qrow + hh:qrow + hd, bcol:bcol + s]))
                            nc.sync.dma_start(out=sw[hh:, :], in_=bc(src[qrow:qrow + hh, bcol:bcol + s]))
                            tmp = b_tmp.tile([hd, s], mmdt, name="rope_tmp", tag="rope_tmp")
                            nc.vector.tensor_mul(tmp[:], tl[:], cos_sb[:, bcol:bcol + s])
                            nc.vector.tensor_mul(sw[:], sw[:], sin_sb[:, bcol:bcol + s])
                            nc.vector.tensor_add(tl[:], tmp[:], sw[:])
                            rot[nm] = tl
                        qt_sb, kt_sb = rot["q"], rot["k"]

                        vts = {}
                        for jq in range(s // qb):
                            kbs = [
                                (k, cls) for k, cls in enumerate(cfg.block_map[jq])
                                if cls != "skip"
                            ]
                            ctx_ps = b_cps.tile([hd, qb], dt.float32, name="ctx_ps", tag="ctx_ps")
                            e_acc = b_acc.tile([KB, qb], mmdt, name="e_acc", tag="e_acc")
                            rhs_q = qt_sb[:, jq * qb:(jq + 1) * qb]
                            for i, (k, cls) in enumerate(kbs):
                                if k not in vts:
                                    vt = b_v.tile([KB, hd], mmdt, name="v_t", tag="v_t")
                                    nc.sync.dma_start(
                                        out=vt[:],
                                        in_=bc(v_dram[bcol + k * KB:bcol + (k + 1) * KB,
                                                      qrow:qrow + hd]),
                                    )
                                    vts[k] = vt
                                st = b_sps.tile([KB, qb], dt.float32, name="st_ps", tag="st_ps")
                                mm(st[:], kt_sb[:, k * KB:(k + 1) * KB], rhs_q,
                                   start=True, stop=True)
                                if cls != "free":
                                    nc.vector.tensor_add(st[:], st[:], mask_sb[cls][:])
                                pt = b_p.tile([KB, qb], mmdt, name="pt_sb", tag="pt_sb")
                                nc.scalar.activation(
                                    pt[:], st[:], mybir.ActivationFunctionType.Exp,
                                    scale=float(SCALE),
                                )
                                if i == 0:
                                    nc.vector.tensor_copy(e_acc[:], pt[:])
                                else:
                                    nc.vector.tensor_add(e_acc[:], e_acc[:], pt[:])
                                mm(ctx_ps[:], vts[k][:], pt[:],
                                   start=(i == 0), stop=(i == len(kbs) - 1))
                            dn = b_dps.tile([128, qb], dt.float32, name="dn_ps", tag="dn_ps")
                            mm(dn[:], ones_sb[:], e_acc[:], start=True, stop=True)
                            rinv = b_acc.tile([128, qb], dt.float32, name="rinv", tag="rinv")
                            nc.vector.reciprocal(rinv[:], dn[:])
                            ctx_sb = b_acc.tile([hd, qb], dt.float32, name="ctx_sb", tag="ctx_sb")
                            nc.vector.tensor_mul(ctx_sb[:], ctx_ps[:], rinv[:hd, :])
                            dest = bb * (s // qb) + jq
                            nc.sync.dma_start(
                                out=a2a_in[dest, qrow:qrow + hd, :], in_=ctx_sb[:]
                            )

            # ---------------- stage C: all-to-all + output projection ----------------
            nc.gpsimd.collective_compute(
                "AllToAll",
                mybir.AluOpType.bypass,
                replica_groups=[list(range(cfg.ncores))],
                ins=[a2a_in.opt()],
                outs=[a2a_out.opt()],
            )
            ctx_full = a2a_out.rearrange("c f n -> (c f) n")  # [H, NB]
            with (
                tc.tile_pool(name="c_ctx", bufs=1) as c_ctx,
                tc.tile_pool(name="c_wd", bufs=nkt + 8) as c_wd,
                tc.tile_pool(name="c_o", bufs=3) as c_o,
                tc.tile_pool(name="c_ps", bufs=4, space="PSUM") as c_ps,
            ):
                cf_sb = []
                for k in range(nkt):
                    tl = c_ctx.tile([128, nb], mmdt, name=f"cf_sb{k}")
                    nc.sync.dma_start(out=tl[:], in_=bc(ctx_full[k * 128:(k + 1) * 128, :]))
                    cf_sb.append(tl)
                for n in range(h // ob):
                    wds = []
                    for k in range(nkt):
                        wt = c_wd.tile([128, ob], mmdt, name="wd_t", tag="wd_t")
                        nc.sync.dma_start(
                            out=wt[:], in_=bc(wdT[k * 128:(k + 1) * 128, n * ob:(n + 1) * ob])
                        )
                        wds.append(wt)
                    for tsub in range(nb // 128):
                        ps = c_ps.tile([128, ob], dt.float32, name="o_ps", tag="o_ps")
                        for k in range(nkt):
                            mm(ps[:], cf_sb[k][:, tsub * 128:(tsub + 1) * 128], wds[k][:],
                               start=(k == 0), stop=(k == nkt - 1))
                        osb = c_o.tile([128, ob], dt.float32, name="o_sb", tag="o_sb")
                        nc.vector.tensor_copy(osb[:], ps[:])
                        nc.sync.dma_start(
                            out=out[tsub * 128:(tsub + 1) * 128, n * ob:(n + 1) * ob],
                            in_=osb[:],
                        )
    nc.compile()
    return nc


# ------------------------------------------------------------------ executor
def _execute(nc, in_maps, n_cores, n_timed=0):
    """Run the prebuilt Bass graph on the axon-tunneled cores via PJRT.

    Mirrors bass2jax.run_bass_via_pjrt but keeps inputs device-resident and
    (optionally) re-executes the NEFF n_timed times, recording min wall-clock.
    Returns (results, timed_ns_min_or_None).
    """
    import time as _time

    import jax
    from jax.experimental.shard_map import shard_map
    from jax.sharding import Mesh, NamedSharding, PartitionSpec

    from concourse import bass2jax, mybir

    bass2jax.install_neuronx_cc_hook()
    assert nc.dbg_addr is None or not nc.dbg_callbacks

    partition_name = nc.partition_id_tensor.name if nc.partition_id_tensor else None
    in_names, out_names, out_avals, zero_outs = [], [], [], []
    for alloc in nc.m.functions[0].allocations:
        if not isinstance(alloc, mybir.MemoryLocationSet):
            continue
        name = alloc.memorylocations[0].name
        if alloc.kind == "ExternalInput":
            if name != partition_name and name != (nc.dbg_addr.name if nc.dbg_addr else None):
                in_names.append(name)
        elif alloc.kind == "ExternalOutput":
            shape = tuple(alloc.tensor_shape)
            dtype = mybir.dt.np(alloc.dtype)
            out_avals.append(jax.core.ShapedArray(shape, dtype))
            out_names.append(name)
            zero_outs.append(np.zeros(shape, dtype))
    n_params = len(in_names)
    all_in_names = list(in_names) + list(out_names)
    if nc.dbg_addr is not None:
        in_maps = [
            {**m, nc.dbg_addr.name: np.zeros((1, 2), np.uint32)} for m in in_maps
        ]
        all_in_names.append(nc.dbg_addr.name)
        n_dbg = 1
    else:
        n_dbg = 0
    if partition_name is not None:
        all_in_names.append(partition_name)

    def _body(*args):
        operands = list(args)
        if partition_name is not None:
            operands.append(bass2jax.partition_id_tensor())
        outs = bass2jax._bass_exec_p.bind(
            *operands,
            out_avals=tuple(out_avals),
            in_names=tuple(all_in_names),
            out_names=tuple(out_names),
            lowering_input_output_aliases=(),
            sim_require_finite=True,
            sim_require_nnan=True,
            nc=nc,
        )
        return tuple(outs)

    devices = jax.devices()[:n_cores]
    assert len(devices) == n_cores
    mesh = Mesh(np.asarray(devices), ("core",))
    n_ops = n_params + len(out_names) + n_dbg
    sharded = jax.jit(
        shard_map(
            _body,
            mesh=mesh,
            in_specs=(PartitionSpec("core"),) * n_ops,
            out_specs=(PartitionSpec("core"),) * len(out_names),
            check_rep=False,
        ),
        keep_unused=True,
    )
    sh = NamedSharding(mesh, PartitionSpec("core"))
    dev_args = []
    for i, name in enumerate(all_in_names[:n_params]):
        cat = np.concatenate([np.asarray(m[name]) for m in in_maps], axis=0)
        dev_args.append(jax.device_put(cat, sh))
    for z in zero_outs:
        cat = np.zeros((n_cores * z.shape[0], *z.shape[1:]), z.dtype)
        dev_args.append(jax.device_put(cat, sh))
    if n_dbg:
        name = nc.dbg_addr.name
        cat = np.concatenate([np.asarray(m[name]) for m in in_maps], axis=0)
        dev_args.append(jax.device_put(cat, sh))

    out_arrs = sharded(*dev_args)
    jax.block_until_ready(out_arrs)

    timed = None
    if n_timed > 0:
        times = []
        for _ in range(n_timed):
            t0 = _time.perf_counter()
            r = sharded(*dev_args)
            jax.block_until_ready(r)
            times.append(_time.perf_counter() - t0)
        timed = int(min(times) * 1e9)

    results = [
        {
            name: np.asarray(out_arrs[i]).reshape(n_cores, *out_avals[i].shape)[c]
            for i, name in enumerate(out_names)
        }
        for c in range(n_cores)
    ]
    return results, timed


# ------------------------------------------------------------------ entry point
def kernel(hidden_states, Wq, Wk, Wv, Wd, attention_mask, position_ids):
    global LAST_EXEC_NS
    cfg = Config()
    in_maps = prepare(hidden_states, Wq, Wk, Wv, Wd, attention_mask, position_ids, cfg)
    nc = build_graph(cfg)

    n_timed = int(os.environ.get("BASS_KERNEL_TIME", "0"))
    results, timed = _execute(nc, in_maps, cfg.ncores, n_timed=n_timed)
    LAST_EXEC_NS = timed
    outs = [np.asarray(results[i]["out"]) for i in range(cfg.ncores)]
    full = np.concatenate(outs, axis=0).reshape(B, S, H)
    return full.astype(np.float32)


# revision 11
# speedup vs baseline: 102.8808x; 102.8808x over previous
"""Distributed Bass kernel for a dense-transformer attention layer on 8 TRN2 cores.

Strategy (tensor-parallel over heads, Megatron-style, zero on-device transposes):
  - Host: transpose hidden -> xT [H, T]; per-core column-shard of Wq/Wk (rows
    permuted per head so rotary pairs land in partition halves), Wv; full Wd.T.
  - Device, per core (SPMD, 2 heads each):
      A) QKV projections:  Q_T/K_T produced directly in [head_dim, tokens]
         layout (weights stationary, xT streamed);  V in [tokens, head_dim]
         layout (xT stationary).  Spilled to per-batch DRAM scratch with
         batched multi-tile DMAs.
      B) Attention per (head, batch): load Q_T/K_T head tile plus a partition-
         half-swapped copy (DMA), apply RoPE with three partition-aligned DVE
         ops using host cos/sin tables.  Scores computed TRANSPOSED
         S_T[kpos, q] = K_T_tile(lhsT) @ Q_T block, causal block-skipping,
         0/1 mask multiply (GpSimd) after an exp without max-subtraction
         (score magnitudes are tiny), softmax denominators accumulated in two
         parity-split tiles (DVE + GpSimd) and reduced/broadcast by
         ones-matmuls, context accumulated as ctx_T[d,q] += V_tile(lhsT) @ P_T.
      C) Per-head AllToAll of ctx features (overlaps the other head's
         attention and the output projection); out = ctx_T.T @ Wd.T on this
         core's 512-token block.
  - Host: concat the 8 token blocks -> [B, S, H].
"""

import os
import sys
import math
from dataclasses import dataclass, field

import numpy as np

sys.path.insert(0, "/opt/trn_rl_repo")

# ---------------------------------------------------------------- problem dims
B, S, H, NH = 2, 2048, 2048, 16
HD = H // NH  # 128
NCORES = 8
ROPE_BASE = 10000.0
SCALE = 1.0 / math.sqrt(HD)

KB = 128  # key-block (kpos per score tile)
QB = 512  # query-block (free dim of score tiles) == tokens per core block
OB = 512  # output-projection n-chunk

LAST_EXEC_NS = None


@dataclass
class Config:
    b: int = B
    s: int = S
    h: int = H
    nh: int = NH
    ncores: int = NCORES
    qb: int = QB
    ob: int = OB
    qbp: int = 512  # token-chunk width for the projection stage
    use_fp32r: bool = True
    # block_map[jq][kb] = "skip" | "free" | mask-tile index (int)
    block_map: list = field(default_factory=list)
    n_bias: int = 0

    @property
    def hd(self):
        return self.h // self.nh

    @property
    def t(self):
        return self.b * self.s

    @property
    def hpc(self):  # heads per core
        return self.nh // self.ncores

    @property
    def f(self):  # features per core
        return self.hpc * self.hd

    @property
    def nb(self):  # tokens per core output block
        return self.t // self.ncores


def classify_blocks(mask2d: np.ndarray, qb: int, kb: int):
    """mask2d: [S, S] bool, True = masked out.  Returns (block_map, mul_tiles).

    block_map[jq][k] in {"skip", "free", int}; mul_tiles[i] is [KB, QB] f32
    (transposed: [kpos, q]) with 0.0 where masked, 1.0 where kept.
    """
    s = mask2d.shape[0]
    tiles = []
    keys = {}
    block_map = []
    for jq in range(s // qb):
        row = []
        for k in range(s // kb):
            sub = mask2d[jq * qb:(jq + 1) * qb, k * kb:(k + 1) * kb]
            if sub.all():
                row.append("skip")
            elif not sub.any():
                row.append("free")
            else:
                tile = np.where(sub.T, np.float32(0.0), np.float32(1.0))
                key = tile.tobytes()
                if key not in keys:
                    keys[key] = len(tiles)
                    tiles.append(np.ascontiguousarray(tile, np.float32))
                row.append(keys[key])
        assert any(c != "skip" for c in row), "fully-masked query block"
        block_map.append(row)
    return block_map, tiles


# ------------------------------------------------------------------ host sharding
def prepare(hidden_states, Wq, Wk, Wv, Wd, attention_mask, position_ids, cfg):
    s, h, nh, hd = cfg.s, cfg.h, cfg.nh, cfg.hd
    t = cfg.t

    x = np.ascontiguousarray(np.asarray(hidden_states, np.float32).reshape(t, h))
    xT = np.ascontiguousarray(x.T)  # [H, T]

    # per-head pair permutation: [0,2,...,hd-2, 1,3,...,hd-1]
    pp = np.concatenate([np.arange(0, hd, 2), np.arange(1, hd, 2)])
    perm = np.concatenate([hh * hd + pp for hh in range(nh)])

    WqP = np.asarray(Wq, np.float32)[perm]
    WkP = np.asarray(Wk, np.float32)[perm]
    Wv_ = np.asarray(Wv, np.float32)
    WdT = np.ascontiguousarray(np.asarray(Wd, np.float32).T)  # [H(in f), H(out)]

    inv_freq = (1.0 / (ROPE_BASE ** (np.arange(0, hd, 2, dtype=np.float32) / np.float32(hd)))).astype(np.float32)
    pos = np.asarray(position_ids).astype(np.float32).reshape(t)  # [T]
    ang = pos[None, :] * inv_freq[:, None]  # [hd/2, T]
    cos = np.cos(ang).astype(np.float32)
    sin = np.sin(ang).astype(np.float32)
    cosT = np.ascontiguousarray(np.concatenate([cos, cos], axis=0))  # [hd, T]
    sinT = np.ascontiguousarray(np.concatenate([-sin, sin], axis=0))  # [hd, T]

    mask2d = np.asarray(attention_mask).reshape(s, s).astype(bool)
    cfg.block_map, mul_tiles = classify_blocks(mask2d, cfg.qb, KB)
    cfg.n_bias = len(mul_tiles)
    maskb = None
    if cfg.n_bias:
        maskb = np.ascontiguousarray(np.stack(mul_tiles, axis=0))

    in_maps = []
    f = cfg.f
    for c in range(cfg.ncores):
        m = {
            "xT": xT,
            "wqT": np.ascontiguousarray(WqP[c * f:(c + 1) * f].T),  # [H, F]
            "wkT": np.ascontiguousarray(WkP[c * f:(c + 1) * f].T),
            "wvT": np.ascontiguousarray(Wv_[c * f:(c + 1) * f].T),
            "wdT": WdT,
            "cosT": cosT,
            "sinT": sinT,
            "ones": np.ones((128, 128), np.float32),
        }
        if cfg.n_bias:
            m["maskb"] = maskb
        in_maps.append(m)
    return in_maps


# ------------------------------------------------------------------ graph builder
def build_graph(cfg, repeat=1):
    import concourse.tile as tile
    from concourse import bacc, mybir

    dt = mybir.dt
    mmdt = dt.float32r if cfg.use_fp32r else dt.float32

    b, s, h = cfg.b, cfg.s, cfg.h
    t, f, hd, hpc = cfg.t, cfg.f, cfg.hd, cfg.hpc
    nb, qb, ob, qbp = cfg.nb, cfg.qb, cfg.ob, cfg.qbp
    nkt = h // 128           # k-tiles over hidden dim
    npc = t // qbp           # token chunks in projection stage
    nts = qbp // 128         # t-subtiles per projection chunk
    nsk = s // 128           # key tiles per batch
    hh = hd // 2
    assert qb == nb and hd == 128

    nc = bacc.Bacc(None, target_bir_lowering=False)

    xT = nc.declare_dram_parameter("xT", [h, t], dt.float32, isOutput=False)
    wqT = nc.declare_dram_parameter("wqT", [h, f], dt.float32, isOutput=False)
    wkT = nc.declare_dram_parameter("wkT", [h, f], dt.float32, isOutput=False)
    wvT = nc.declare_dram_parameter("wvT", [h, f], dt.float32, isOutput=False)
    wdT = nc.declare_dram_parameter("wdT", [h, h], dt.float32, isOutput=False)
    cosT = nc.declare_dram_parameter("cosT", [hd, t], dt.float32, isOutput=False)
    sinT = nc.declare_dram_parameter("sinT", [hd, t], dt.float32, isOutput=False)
    ones_d = nc.declare_dram_parameter("ones", [128, 128], dt.float32, isOutput=False)
    if cfg.n_bias:
        maskb = nc.declare_dram_parameter("maskb", [cfg.n_bias, KB, qb], dt.float32, isOutput=False)
    out = nc.declare_dram_parameter("out", [nb, h], dt.float32, isOutput=True)

    def mm(o, lhsT, rhs, start, stop):
        nc.tensor.matmul(o, lhsT, rhs, start=start, stop=stop)

    def bc(ap):  # view a float32 DRAM AP as the matmul dtype
        return ap.bitcast(mmdt)

    xT3 = xT.ap().rearrange("(k p) t -> p k t", p=128)       # [128, nkt, T]
    wdT3 = wdT.ap().rearrange("(k p) o -> p k o", p=128)     # [128, nkt, H]
    out3 = out.ap().rearrange("(r p) o -> p r o", p=128)     # [128, nb/128, H]

    with tile.TileContext(nc) as tc:
        with (
            tc.tile_pool(name="persist", bufs=1) as persist,
            tc.tile_pool(name="dram", bufs=1, space="DRAM") as dram,
            tc.tile_pool(name="psum", bufs=1, space="PSUM") as psum,
        ):
            ones_sb = persist.tile([128, 128], mmdt, name="ones_sb")
            nc.sync.dma_start(out=ones_sb[:], in_=bc(ones_d[:, :]))
            mask_sb = []
            for i in range(cfg.n_bias):
                mt = persist.tile([KB, qb], mmdt, name=f"mask_sb{i}")
                nc.sync.dma_start(out=mt[:], in_=bc(maskb[i, :, :]))
                mask_sb.append(mt)

            for _rep in range(repeat):
                qt_b = [dram.tile([f, s], dt.float32, name=f"qt_b{bb}", tag=f"qt_b{bb}") for bb in range(b)]
                kt_b = [dram.tile([f, s], dt.float32, name=f"kt_b{bb}", tag=f"kt_b{bb}") for bb in range(b)]
                v_b = [dram.tile([s, f], dt.float32, name=f"v_b{bb}", tag=f"v_b{bb}") for bb in range(b)]
                a2a_in = [dram.tile([cfg.ncores, hd, nb], dt.float32, name=f"a2a_in{hi}", tag=f"a2a_in{hi}")
                          for hi in range(hpc)]
                a2a_out = [dram.tile([cfg.ncores, hd, nb], dt.float32, name=f"a2a_out{hi}", tag=f"a2a_out{hi}")
                           for hi in range(hpc)]

                # ---------------- stage A: QKV projections ----------------
                with (
                    tc.tile_pool(name="a_w", bufs=1) as a_w,
                    tc.tile_pool(name="a_x", bufs=2) as a_x,
                    tc.tile_pool(name="a_ep", bufs=3) as a_ep,
                ):
                    wq_sb = a_w.tile([128, nkt, f], mmdt, name="wq_sb", tag="wq_sb")
                    wk_sb = a_w.tile([128, nkt, f], mmdt, name="wk_sb", tag="wk_sb")
                    wv_sb = a_w.tile([128, nkt, f], mmdt, name="wv_sb", tag="wv_sb")
                    for tl_, src in ((wq_sb, wqT), (wk_sb, wkT), (wv_sb, wvT)):
                        nc.sync.dma_start(
                            out=tl_[:],
                            in_=bc(src.ap().rearrange("(k p) f -> p k f", p=128)),
                        )

                    for c in range(npc):
                        c0 = c * qbp
                        bq, c0r = c0 // s, c0 % s
                        xsl = a_x.tile([128, nkt, qbp], mmdt, name="xsl", tag="xsl")
                        nc.sync.dma_start(out=xsl[:], in_=bc(xT3[:, :, c0:c0 + qbp]))

                        for w_sb, dst in ((wq_sb, qt_b[bq]), (wk_sb, kt_b[bq])):
                            qkcat = a_ep.tile([128, hpc, qbp], dt.float32,
                                              name="qkcat", tag="qkcat")
                            for ft in range(hpc):
                                ps = psum.tile([128, qbp], dt.float32,
                                               name="proj_ps", tag="proj_ps", bufs=2)
                                for k in range(nkt):
                                    mm(ps[:], w_sb[:, k, ft * 128:(ft + 1) * 128],
                                       xsl[:, k, :], start=(k == 0), stop=(k == nkt - 1))
                                nc.vector.tensor_copy(qkcat[:, ft, :], ps[:])
                            nc.sync.dma_start(
                                out=dst.rearrange("(ft p) sdim -> p ft sdim", p=128)[:, :, c0r:c0r + qbp],
                                in_=qkcat[:],
                            )

                        vcat = a_ep.tile([128, nts, f], dt.float32, name="vcat", tag="qkcat")
                        for ts in range(nts):
                            ps = psum.tile([128, f], dt.float32,
                                           name="v_ps", tag="proj_ps", bufs=2)
                            for k in range(nkt):
                                mm(ps[:], xsl[:, k, ts * 128:(ts + 1) * 128],
                                   wv_sb[:, k, :], start=(k == 0), stop=(k == nkt - 1))
                            nc.vector.tensor_copy(vcat[:, ts, :], ps[:])
                        r0 = c0r // 128
                        nc.sync.dma_start(
                            out=v_b[bq].rearrange("(r p) f -> p r f", p=128)[:, r0:r0 + nts, :],
                            in_=vcat[:],
                        )

                # ---------------- stage B: RoPE + attention (head-outer) ----------------
                with (
                    tc.tile_pool(name="b_cs", bufs=1) as b_cs,
                    tc.tile_pool(name="b_qk", bufs=2) as b_qk,
                    tc.tile_pool(name="b_sw", bufs=2) as b_sw,
                    tc.tile_pool(name="b_tmp", bufs=2) as b_tmp,
                    tc.tile_pool(name="b_v", bufs=2) as b_v,
                    tc.tile_pool(name="b_p", bufs=6) as b_p,
                    tc.tile_pool(name="b_acc", bufs=3) as b_acc,
                ):
                    cos_sb = b_cs.tile([hd, t], mmdt, name="cos_sb", tag="cos_sb")
                    sin_sb = b_cs.tile([hd, t], mmdt, name="sin_sb", tag="sin_sb")
                    nc.sync.dma_start(out=cos_sb[:], in_=bc(cosT[:, :]))
                    nc.sync.dma_start(out=sin_sb[:], in_=bc(sinT[:, :]))

                    for hi in range(hpc):
                        qrow = hi * hd
                        for bb in range(b):
                            bcol = bb * s
                            rot = {}
                            for src, nm in ((qt_b[bb], "q"), (kt_b[bb], "k")):
                                tl_ = b_qk.tile([hd, s], mmdt, name=f"{nm}t_sb", tag=f"{nm}t_sb")
                                sw = b_sw.tile([hd, s], mmdt, name=f"{nm}t_sw", tag=f"{nm}t_sw")
                                nc.sync.dma_start(out=tl_[:], in_=bc(src[qrow:qrow + hd, :]))
                                nc.sync.dma_start(out=sw[:hh, :], in_=bc(src[qrow + hh:qrow + hd, :]))
                                nc.sync.dma_start(out=sw[hh:, :], in_=bc(src[qrow:qrow + hh, :]))
                                tmp = b_tmp.tile([hd, s], mmdt, name="rope_tmp", tag="rope_tmp")
                                nc.vector.tensor_mul(tmp[:], tl_[:], cos_sb[:, bcol:bcol + s])
                                nc.vector.tensor_mul(sw[:], sw[:], sin_sb[:, bcol:bcol + s])
                                nc.vector.tensor_add(tl_[:], tmp[:], sw[:])
                                rot[nm] = tl_
                            qt_sb, kt_sb = rot["q"], rot["k"]

                            vz = b_v.tile([128, nsk, hd], mmdt, name="v_sb", tag="v_sb")
                            nc.sync.dma_start(
                                out=vz[:],
                                in_=bc(v_b[bb].rearrange("(k p) f -> p k f", p=128)[:, :, qrow:qrow + hd]),
                            )

                            for jq in range(s // qb):
                                kbs = [(k, cls) for k, cls in enumerate(cfg.block_map[jq])
                                       if cls != "skip"]
                                ctx_ps = psum.tile([hd, qb], dt.float32,
                                                   name="ctx_ps", tag="ctx_ps", bufs=1)
                                dn = psum.tile([1, qb], dt.float32,
                                               name="dn_ps", tag="dn_ps", bufs=1)
                                rhs_q = qt_sb[:, jq * qb:(jq + 1) * qb]
                                for i, (k, cls) in enumerate(kbs):
                                    st = psum.tile([KB, qb], dt.float32,
                                                   name="st_ps", tag="st_ps", bufs=2)
                                    mm(st[:], kt_sb[:, k * KB:(k + 1) * KB], rhs_q,
                                       start=True, stop=True)
                                    pt = b_p.tile([KB, qb], mmdt, name="pt_sb", tag="pt_sb")
                                    nc.scalar.activation(
                                        pt[:], st[:], mybir.ActivationFunctionType.Exp,
                                        scale=float(SCALE),
                                    )
                                    if cls != "free":
                                        nc.vector.tensor_mul(pt[:], pt[:], mask_sb[cls][:])
                                    mm(dn[:], ones_sb[:, 0:1], pt[:],
                                       start=(i == 0), stop=(i == len(kbs) - 1))
                                    mm(ctx_ps[:], vz[:, k, :], pt[:],
                                       start=(i == 0), stop=(i == len(kbs) - 1))
                                dsb = b_acc.tile([1, qb], mmdt, name="dsb", tag="dsb")
                                nc.vector.tensor_copy(dsb[:], dn[:])
                                bps = psum.tile([128, qb], dt.float32,
                                                name="bps", tag="dn_ps", bufs=1)
                                mm(bps[:], ones_sb[0:1, :], dsb[:], start=True, stop=True)
                                rinv = b_acc.tile([128, qb], dt.float32, name="rinv", tag="rinv")
                                nc.vector.reciprocal(rinv[:], bps[:])
                                ctx_sb = b_acc.tile([hd, qb], dt.float32, name="ctx_sb", tag="ctx_sb")
                                nc.vector.tensor_mul(ctx_sb[:], ctx_ps[:], rinv[:hd, :])
                                dest = bb * (s // qb) + jq
                                nc.sync.dma_start(out=a2a_in[hi][dest, :, :], in_=ctx_sb[:])

                        nc.gpsimd.collective_compute(
                            "AllToAll",
                            mybir.AluOpType.bypass,
                            replica_groups=[list(range(cfg.ncores))],
                            ins=[a2a_in[hi].opt()],
                            outs=[a2a_out[hi].opt()],
                        )

                # ---------------- stage C: output projection ----------------
                with (
                    tc.tile_pool(name="c_ctx", bufs=1) as c_ctx,
                    tc.tile_pool(name="c_wd", bufs=2) as c_wd,
                    tc.tile_pool(name="c_o", bufs=2) as c_o,
                ):
                    cf_sb = []
                    for hi in range(hpc):
                        tl_ = c_ctx.tile([128, cfg.ncores, nb], mmdt, name=f"cf_sb{hi}", tag=f"cf_sb{hi}")
                        nc.sync.dma_start(
                            out=tl_[:],
                            in_=bc(a2a_out[hi].rearrange("c p n -> p c n")),
                        )
                        cf_sb.append(tl_)
                    for n in range(h // ob):
                        wda = c_wd.tile([128, nkt, ob], mmdt, name="wda", tag="wda")
                        nc.sync.dma_start(out=wda[:], in_=bc(wdT3[:, :, n * ob:(n + 1) * ob]))
                        ocat = c_o.tile([128, nb // 128, ob], dt.float32, name="ocat", tag="ocat")
                        for tsub in range(nb // 128):
                            pss = []
                            for hi in range(hpc):
                                ps = psum.tile([128, ob], dt.float32,
                                               name=f"o_ps{hi}", tag=f"o_ps{hi}", bufs=1)
                                for cc in range(cfg.ncores):
                                    kf = cc * hpc + hi
                                    mm(ps[:], cf_sb[hi][:, cc, tsub * 128:(tsub + 1) * 128],
                                       wda[:, kf, :], start=(cc == 0),
                                       stop=(cc == cfg.ncores - 1))
                                pss.append(ps)
                            if len(pss) == 1:
                                nc.vector.tensor_copy(ocat[:, tsub, :], pss[0][:])
                            else:
                                hsum = c_o.tile([128, ob], dt.float32, name="hsum", tag="hsum")
                                nc.vector.tensor_copy(hsum[:], pss[0][:])
                                for ps2 in pss[1:]:
                                    nc.vector.tensor_add(ocat[:, tsub, :], hsum[:], ps2[:])
                        nc.sync.dma_start(
                            out=out3[:, :, n * ob:(n + 1) * ob], in_=ocat[:]
                        )
    nc.compile()
    return nc


# ------------------------------------------------------------------ executor
def _prepare_exec_full(nc, in_maps, n_cores):
    """Build the sharded jit callable + device-resident args for nc."""
    import jax
    from jax.experimental.shard_map import shard_map
    from jax.sharding import Mesh, NamedSharding, PartitionSpec

    from concourse import bass2jax, mybir

    bass2jax.install_neuronx_cc_hook()
    assert nc.dbg_addr is None or not nc.dbg_callbacks

    partition_name = nc.partition_id_tensor.name if nc.partition_id_tensor else None
    in_names, out_names, out_avals, zero_outs = [], [], [], []
    for alloc in nc.m.functions[0].allocations:
        if not isinstance(alloc, mybir.MemoryLocationSet):
            continue
        name = alloc.memorylocations[0].name
        if alloc.kind == "ExternalInput":
            if name != partition_name and name != (nc.dbg_addr.name if nc.dbg_addr else None):
                in_names.append(name)
        elif alloc.kind == "ExternalOutput":
            shape = tuple(alloc.tensor_shape)
            dtype = mybir.dt.np(alloc.dtype)
            out_avals.append(jax.core.ShapedArray(shape, dtype))
            out_names.append(name)
            zero_outs.append(np.zeros(shape, dtype))
    n_params = len(in_names)
    all_in_names = list(in_names) + list(out_names)
    if nc.dbg_addr is not None:
        in_maps = [
            {**m, nc.dbg_addr.name: np.zeros((1, 2), np.uint32)} for m in in_maps
        ]
        all_in_names.append(nc.dbg_addr.name)
        n_dbg = 1
    else:
        n_dbg = 0
    if partition_name is not None:
        all_in_names.append(partition_name)

    def _body(*args):
        operands = list(args)
        if partition_name is not None:
            operands.append(bass2jax.partition_id_tensor())
        outs = bass2jax._bass_exec_p.bind(
            *operands,
            out_avals=tuple(out_avals),
            in_names=tuple(all_in_names),
            out_names=tuple(out_names),
            lowering_input_output_aliases=(),
            sim_require_finite=True,
            sim_require_nnan=True,
            nc=nc,
        )
        return tuple(outs)

    devices = jax.devices()[:n_cores]
    assert len(devices) == n_cores
    mesh = Mesh(np.asarray(devices), ("core",))
    n_ops = n_params + len(out_names) + n_dbg
    sharded = jax.jit(
        shard_map(
            _body,
            mesh=mesh,
            in_specs=(PartitionSpec("core"),) * n_ops,
            out_specs=(PartitionSpec("core"),) * len(out_names),
            check_rep=False,
        ),
        keep_unused=True,
    )
    sh = NamedSharding(mesh, PartitionSpec("core"))
    dev_args = []
    for i, name in enumerate(all_in_names[:n_params]):
        cat = np.concatenate([np.asarray(m[name]) for m in in_maps], axis=0)
        dev_args.append(jax.device_put(cat, sh))
    for z in zero_outs:
        cat = np.zeros((n_cores * z.shape[0], *z.shape[1:]), z.dtype)
        dev_args.append(jax.device_put(cat, sh))
    if n_dbg:
        name = nc.dbg_addr.name
        cat = np.concatenate([np.asarray(m[name]) for m in in_maps], axis=0)
        dev_args.append(jax.device_put(cat, sh))
    return sharded, dev_args, out_names, out_avals


def _prepare_exec(nc, in_maps, n_cores):
    fn, args, _, _ = _prepare_exec_full(nc, in_maps, n_cores)
    return fn, args


def _execute(nc, in_maps, n_cores, n_timed=0):
    import time as _time

    import jax

    sharded, dev_args, out_names, out_avals = _prepare_exec_full(nc, in_maps, n_cores)
    out_arrs = sharded(*dev_args)
    jax.block_until_ready(out_arrs)

    timed = None
    if n_timed > 0:
        times = []
        for _ in range(n_timed):
            t0 = _time.perf_counter()
            r = sharded(*dev_args)
            jax.block_until_ready(r)
            times.append(_time.perf_counter() - t0)
        timed = int(min(times) * 1e9)

    results = [
        {
            name: np.asarray(out_arrs[i]).reshape(n_cores, *out_avals[i].shape)[c]
            for i, name in enumerate(out_names)
        }
        for c in range(n_cores)
    ]
    return results, timed


# ------------------------------------------------------------------ entry point
def kernel(hidden_states, Wq, Wk, Wv, Wd, attention_mask, position_ids):
    global LAST_EXEC_NS
    cfg = Config()
    in_maps = prepare(hidden_states, Wq, Wk, Wv, Wd, attention_mask, position_ids, cfg)
    nc = build_graph(cfg)

    n_timed = int(os.environ.get("BASS_KERNEL_TIME", "0"))
    results, timed = _execute(nc, in_maps, cfg.ncores, n_timed=n_timed)
    LAST_EXEC_NS = timed
    outs = [np.asarray(results[i]["out"]) for i in range(cfg.ncores)]
    full = np.concatenate(outs, axis=0).reshape(B, S, H)
    return full.astype(np.float32)
